# revision 1
# baseline (speedup 1.0000x reference)
"""CodeSwitchLoss Trainium2 kernel (8-core data-parallel).

Math (see reference): V = l2norm rows of the stack [e, k, etk, kte] (4096 x 1024),
S = V @ V.T, E = exp(10*S).
Per anchor row r=(a,i):
  rowsum[r]   = sum_c E[r,c]
  d_b[r]      = E[r, col(b,i)]  (same-sample entries, b=0..3)
  pos[r]      = sum_{b != a} d_b[r]
  denom[r]    = rowsum[r] - d_a[r]          (= pos + neg)
  contrastive = log(denom) - log(pos)
plus cs regularization on normalized rows; total = (sum contrastive + 0.5*sum reg)/B.

Sharding: batch samples split 8 ways. Each core gets the full embedding set,
rolled so its own 128 samples come first; it computes the 512 anchor rows
(4 versions x 128 samples) against all 4096 columns. Scalar partials summed on
host. The roll makes all per-core slice offsets compile-time constants, so one
NEFF serves all 8 cores.

Device layout: matmul contracts over D on partitions, so the host ships the
embeddings pre-transposed (raw bf16, [d, s] per version); the device computes
column norms with an all-ones stationary matmul over the squared tiles (which
both partition-sums and broadcasts ss across partitions), then scales columns
in place. No on-device transposes. Per-sample norms for the cs-regularization
are recovered from the broadcast rows via an identity-mask reduce.
"""

import numpy as np
import ml_dtypes

B = 1024
D = 1024
P = 128
NV = 4
NC_CORES = 8
CHUNK = B // NC_CORES  # 128 samples per core
KCH = D // P  # 8 k-chunks
NT = 512  # matmul free-dim tile (one PSUM bank)
INV_T = 10.0  # 1 / TEMPERATURE

_compiled = {}


def _build_kernel(sq_eng="vector", scale_eng="vector", rnb_bf16=True, sq_split=True, builds_first=False, ps_bufs=6, nrm_bufs=2, psum2=False, exp_sbuf=True, halves=False, colmul=True, diag_eng="vector", cs_eng="vector", hybrid_scale=False, sq_fold=8, fp8=False, drow=False, cast_sq=False, esb_bufs=3, sq_bufs=2, wb_first=False, wb_copy=False, wload=1, cs_early=False):
    from contextlib import ExitStack

    import concourse.bass as bass
    import concourse.tile as tile
    from concourse import bacc, mybir

    fp32 = mybir.dt.float32
    bf16 = mybir.dt.bfloat16
    AX = mybir.AxisListType
    ALU = mybir.AluOpType
    ACTF = mybir.ActivationFunctionType

    nc = bacc.Bacc(
        "TRN2",
        target_bir_lowering=False,
        debug=False,
        enable_asserts=False,
        num_devices=NC_CORES,
    )
    emb_dt = mybir.dt.float8e4 if fp8 else bf16
    # pre-transposed raw embeddings: embT[v*D + d, s] = V_v[s, d]
    embT = nc.dram_tensor("embT", [NV * D, B], emb_dt, kind="ExternalInput").ap()
    # natural-layout raw rows of this core's own chunk (for cs reg)
    csrows = nc.dram_tensor("csrows", [NV * P, D], bf16, kind="ExternalInput").ap()
    ratios = nc.dram_tensor("ratios", [P, 1], fp32, kind="ExternalInput").ap()
    eye_d = nc.dram_tensor("eye", [P, P], fp32, kind="ExternalInput").ap()
    out_d = nc.dram_tensor("out", [P, 1], fp32, kind="ExternalOutput").ap()

    with tile.TileContext(nc) as tc, ExitStack() as ctx:
        consts = ctx.enter_context(tc.tile_pool(name="consts", bufs=1))
        wpool = ctx.enter_context(tc.tile_pool(name="w", bufs=1))
        sq_p = ctx.enter_context(tc.tile_pool(name="sq", bufs=sq_bufs))
        rnb_p = ctx.enter_context(tc.tile_pool(name="rnb", bufs=1))
        csx_p = ctx.enter_context(tc.tile_pool(name="csx", bufs=1))
        csn_p = ctx.enter_context(tc.tile_pool(name="csn", bufs=1))
        scr_p = ctx.enter_context(tc.tile_pool(name="scr", bufs=1))
        fold_p = ctx.enter_context(tc.tile_pool(name="fold", bufs=2))
        psum_p = ctx.enter_context(tc.tile_pool(name="psum", bufs=ps_bufs, space="PSUM"))
        esb_p = ctx.enter_context(tc.tile_pool(name="esb", bufs=esb_bufs))
        nrm_ps = ctx.enter_context(tc.tile_pool(name="nrm_ps", bufs=nrm_bufs, space="PSUM"))
        acc_p = ctx.enter_context(tc.tile_pool(name="acc", bufs=1))
        dscr_p = ctx.enter_context(tc.tile_pool(name="dscr", bufs=3))
        fin_p = ctx.enter_context(tc.tile_pool(name="fin", bufs=1))

        eye_sb = consts.tile([P, P], fp32, tag="eye")
        nc.sync.dma_start(out=eye_sb, in_=eye_d)
        r_sb = consts.tile([P, 1], fp32, tag="ratios")
        nc.sync.dma_start(out=r_sb, in_=ratios)
        ones_sb = consts.tile([P, P], bf16, tag="ones")
        nc.vector.memset(ones_sb, 1.0)

        # W[v][p, m, s] = V_v[s, m*128+p] — raw on load, normalized in place
        W = [
            wpool.tile([P, KCH, B], emb_dt, tag=f"w{v}", name=f"w{v}")
            for v in range(NV)
        ]
        rnb_dt = bf16 if (rnb_bf16 and not colmul) else fp32
        rnb = [
            rnb_p.tile([P, B], rnb_dt, tag=f"rnb{v}", name=f"rnb{v}")
            for v in range(NV)
        ]
        rncol = [
            fin_p.tile([P, 1], fp32, tag=f"rncol{v}", name=f"rncol{v}")
            for v in range(NV)
        ]
        rncol10 = [
            fin_p.tile([P, 1], fp32, tag=f"rncol10_{v}", name=f"rncol10_{v}")
            for v in range(NV)
        ]
        if psum2:
            rp_all = acc_p.tile([P, NV, NV], fp32, tag="rp_all", name="rp_all")
            rowparts = [rp_all[:, a, :] for a in range(NV)]
        else:
            rp_all = None
            rowparts = [
                acc_p.tile([P, KCH], fp32, tag=f"rp{a}", name=f"rp{a}")
                for a in range(NV)
            ]
        dvals = acc_p.tile([P, NV * NV], fp32, tag="dvals")  # [:, a*NV + b]

        wb_pre = {}
        if cast_sq and wb_first:
            # hoist all SWDGE cast-DMAs so later versions' shadows land early
            for v in range(NV):
                wb = sq_p.tile([P, KCH, B], bf16, tag=f"wbp{v}", name=f"wbp{v}")
                for m in range(KCH):
                    nc.gpsimd.dma_start(
                        out=wb[:, m, :],
                        in_=embT[v * D + m * P : v * D + (m + 1) * P, :],
                    )
                wb_pre[v] = wb

        def build_w(v):
            # load raw transposed chunks (wload chunks per DMA)
            for m in range(0, KCH, wload):
                nc.sync.dma_start(
                    out=W[v][:, m : m + wload, :],
                    in_=embT[v * D + m * P : v * D + (m + wload) * P, :].rearrange(
                        "(mm p) s -> p mm s", p=P
                    ),
                )
            if wb_copy:
                # bf16 shadow via Pool cast-copy (1-input ops run ~line-rate
                # on GpSimd) — keeps the SDMA rings free of the 12MB shadow
                wb = sq_p.tile([P, KCH, B], bf16, tag="wb", name="wb")
                for m in range(KCH):
                    nc.gpsimd.tensor_copy(wb[:, m, :], W[v][:, m, :])
                sq_src = wb
            elif cast_sq and wb_first:
                sq_src = wb_pre[v]
            elif cast_sq:
                # bf16 shadow copy via SWDGE cast-DMA; squares run at bf16 rates
                wb = sq_p.tile([P, KCH, B], bf16, tag="wb", name="wb")
                for m in range(KCH):
                    nc.gpsimd.dma_start(
                        out=wb[:, m, :],
                        in_=embT[v * D + m * P : v * D + (m + 1) * P, :],
                    )
                sq_src = wb
            else:
                sq_src = W[v]
            # squared tile (bf16) for the norm matmul
            sq = sq_p.tile([P, KCH, B], bf16, tag="sq", name="sq")
            if sq_split:
                for m in range(KCH):
                    getattr(nc, sq_eng).tensor_mul(sq[:, m, :], sq_src[:, m, :], sq_src[:, m, :])
            else:
                getattr(nc, sq_eng).tensor_mul(sq, sq_src, sq_src)
            # fold squared chunks pairwise (bf16 adds) to cut norm matmuls
            folded = sq
            nfold = KCH
            while nfold > sq_fold:
                nxt = fold_p.tile([P, nfold // 2, B], bf16, tag=f"sqf{nfold//2}",
                                  name=f"sqf{nfold//2}")
                for q in range(nfold // 2):
                    getattr(nc, sq_eng).tensor_add(
                        nxt[:, q, :], folded[:, 2 * q, :], folded[:, 2 * q + 1, :]
                    )
                folded = nxt
                nfold //= 2
            # ssb[p, s] = sum_d V_v[s, d]^2, identical on every partition p
            for h in range(B // NT):
                ssb = nrm_ps.tile([P, NT], fp32, tag="ssb", name="ssb")
                for m in range(nfold):
                    nc.tensor.matmul(
                        ssb,
                        ones_sb,
                        folded[:, m, h * NT : (h + 1) * NT],
                        start=(m == 0),
                        stop=(m == nfold - 1),
                    )
                nc.scalar.activation(out=ssb, in_=ssb, func=ACTF.Sqrt)
                if rnb_bf16 and not colmul:
                    nc.vector.reciprocal(out=ssb, in_=ssb)
                    nc.scalar.copy(rnb[v][:, h * NT : (h + 1) * NT], ssb)
                else:
                    nc.vector.reciprocal(
                        out=rnb[v][:, h * NT : (h + 1) * NT], in_=ssb
                    )
            rb = rnb[v]
            if not colmul:
                # normalize columns in place: W[v][p, m, s] *= rnb[v][p, s]
                brd = bass.AP(
                    tensor=rb.tensor,
                    offset=rb.offset,
                    ap=[rb.ap[0], [0, KCH], rb.ap[1]],
                )
                getattr(nc, scale_eng).tensor_mul(W[v], W[v], brd)
            # per-sample 1/norm for this core's own chunk, as a column:
            # rnb rows are identical, so diag(rnb[:, 0:128]) = rn[s], s<128
            nc.vector.scalar_tensor_tensor(
                out=dscr_p.tile([P, P], fp32, tag="dscr", name="dscr"),
                in0=rb[:, 0:P], scalar=1.0, in1=eye_sb,
                op0=ALU.mult, op1=ALU.mult, accum_out=rncol[v],
            )
            if colmul:
                nc.vector.tensor_scalar_mul(rncol10[v], rncol[v], INV_T)

        def build_w_half(v, h):
            hs = slice(h * NT, (h + 1) * NT)
            for m in range(KCH):
                nc.sync.dma_start(
                    out=W[v][:, m, hs],
                    in_=embT[v * D + m * P : v * D + (m + 1) * P, hs],
                )
            sq = sq_p.tile([P, KCH, NT], bf16, tag="sqh", name="sqh")
            nc.vector.tensor_mul(sq, W[v][:, :, hs], W[v][:, :, hs])
            ssb = nrm_ps.tile([P, NT], fp32, tag="ssb", name="ssb")
            for m in range(KCH):
                nc.tensor.matmul(
                    ssb, ones_sb, sq[:, m, :],
                    start=(m == 0), stop=(m == KCH - 1),
                )
            nc.scalar.activation(out=ssb, in_=ssb, func=ACTF.Sqrt)
            nc.vector.reciprocal(out=ssb, in_=ssb)
            nc.scalar.copy(rnb[v][:, hs], ssb)
            rbh = rnb[v][:, hs]
            brd = bass.AP(
                tensor=rbh.tensor, offset=rbh.offset,
                ap=[rbh.ap[0], [0, KCH], rbh.ap[1]],
            )
            nc.vector.tensor_mul(W[v][:, :, hs], W[v][:, :, hs], brd)
            if h == 0:
                nc.vector.scalar_tensor_tensor(
                    out=dscr_p.tile([P, P], fp32, tag="dscr", name="dscr"),
                    in0=rnb[v][:, 0:P], scalar=1.0, in1=eye_sb,
                    op0=ALU.mult, op1=ALU.mult, accum_out=rncol[v],
                )

        def mm_group(n, a):
            v = n // 2
            off = (n % 2) * NT
            ps = psum_p.tile([P, NT], fp32, tag="ps", name="ps")
            if drow:
                for m in range(0, KCH, 2):
                    nc.tensor.matmul(
                        ps,
                        W[a][:, m : m + 2, 0:P],
                        W[v][:, m : m + 2, off : off + NT],
                        start=(m == 0),
                        stop=(m == KCH - 2),
                        perf_mode=mybir.MatmulPerfMode.DoubleRow,
                    )
            else:
                for m in range(KCH):
                    nc.tensor.matmul(
                        ps,
                        W[a][:, m, 0:P],
                        W[v][:, m, off : off + NT],
                        start=(m == 0),
                        stop=(m == KCH - 1),
                    )
            # E = exp(10*S); rowsum partial on the fly
            if colmul:
                # column norms: rnb rows are identical, so this scales col j
                # by 1/|x_j|; row norms ride in on the exp scale AP below
                nc.vector.tensor_mul(ps, ps, rnb[v][:, off : off + NT])
                exp_scale = rncol10[a]
            else:
                exp_scale = INV_T
            if exp_sbuf:
                e_t = esb_p.tile([P, NT], fp32, tag="e_t", name="e_t")
                nc.scalar.activation(
                    out=e_t, in_=ps, func=ACTF.Exp, scale=exp_scale,
                    accum_out=rowparts[a][:, n : n + 1],
                )
                esrc = e_t
            else:
                nc.scalar.activation(
                    out=ps, in_=ps, func=ACTF.Exp, scale=exp_scale,
                    accum_out=rowparts[a][:, n : n + 1],
                )
                esrc = ps
            if n % 2 == 0:
                b = v
                dscr = dscr_p.tile([P, P], fp32, tag="dscr", name="dscr")
                deng = nc.vector if (esrc.space.name == "PSUM") else getattr(nc, diag_eng)
                deng.scalar_tensor_tensor(
                    out=dscr, in0=esrc[:, 0:P], scalar=1.0, in1=eye_sb,
                    op0=ALU.mult, op1=ALU.mult,
                    accum_out=dvals[:, a * NV + b : a * NV + b + 1],
                )

        def mm_group2(v, a):
            # both 512-col halves of version v in one 2-bank psum tile,
            # one colmul + one exp call over 1024 columns
            ps2 = psum_p.tile([P, 2, NT], fp32, tag="ps2", name="ps2")
            for half in range(2):
                if drow:
                    for m in range(0, KCH, 2):
                        nc.tensor.matmul(
                            ps2[:, half, :],
                            W[a][:, m : m + 2, 0:P],
                            W[v][:, m : m + 2, half * NT : (half + 1) * NT],
                            start=(m == 0),
                            stop=(m == KCH - 2),
                            perf_mode=mybir.MatmulPerfMode.DoubleRow,
                        )
                else:
                    for m in range(KCH):
                        nc.tensor.matmul(
                            ps2[:, half, :],
                            W[a][:, m, 0:P],
                            W[v][:, m, half * NT : (half + 1) * NT],
                            start=(m == 0),
                            stop=(m == KCH - 1),
                        )
            psv = ps2.rearrange("p a b -> p (a b)")  # [128, 1024], 2 banks
            if colmul:
                nc.vector.tensor_mul(psv, psv, rnb[v])
                exp_scale = rncol10[a]
            else:
                exp_scale = INV_T
            e2 = esb_p.tile([P, 2 * NT], fp32, tag="e2", name="e2")
            nc.scalar.activation(
                out=e2, in_=psv, func=ACTF.Exp, scale=exp_scale,
                accum_out=rowparts[a][:, v : v + 1],
            )
            dscr = dscr_p.tile([P, P], fp32, tag="dscr", name="dscr")
            nc.vector.scalar_tensor_tensor(
                out=dscr, in0=e2[:, 0:P], scalar=1.0, in1=eye_sb,
                op0=ALU.mult, op1=ALU.mult,
                accum_out=dvals[:, a * NV + v : a * NV + v + 1],
            )

        def emit_cs():
            # ---- cs regularization on own chunk ----
            csn = []
            for vv_ in range(NV):
                cx = csx_p.tile([P, D], bf16, tag=f"csx{vv_}", name=f"csx{vv_}")
                nc.sync.dma_start(out=cx, in_=csrows[vv_ * P : (vv_ + 1) * P, :])
                cv = csn_p.tile([P, D], fp32, tag=f"csn{vv_}", name=f"csn{vv_}")
                nc.vector.tensor_scalar_mul(cv, cx, rncol[vv_])
                csn.append(cv)
            e0, k0, etk0, kte0 = csn
            t1 = scr_p.tile([P, D], fp32, tag="cs_t1")
            getattr(nc, cs_eng).tensor_sub(t1, e0, k0)
            u = scr_p.tile([P, D], fp32, tag="cs_u")
            nc.vector.tensor_scalar_mul(u, t1, r_sb)
            v1 = scr_p.tile([P, D], fp32, tag="cs_v")
            getattr(nc, cs_eng).tensor_sub(v1, etk0, k0)
            d1 = scr_p.tile([P, D], fp32, tag="cs_d")
            getattr(nc, cs_eng).tensor_sub(d1, v1, u)
            sspack = fin_p.tile([P, 2], fp32, tag="sspack")
            dsq = scr_p.tile([P, D], fp32, tag="cs_dsq")
            nc.vector.scalar_tensor_tensor(
                out=dsq, in0=d1, scalar=1.0, in1=d1,
                op0=ALU.mult, op1=ALU.mult, accum_out=sspack[:, 0:1],
            )
            v2 = scr_p.tile([P, D], fp32, tag="cs_v")
            getattr(nc, cs_eng).tensor_sub(v2, kte0, e0)
            d2 = scr_p.tile([P, D], fp32, tag="cs_d")
            getattr(nc, cs_eng).tensor_add(d2, v2, u)
            dsq2 = scr_p.tile([P, D], fp32, tag="cs_dsq")
            nc.vector.scalar_tensor_tensor(
                out=dsq2, in0=d2, scalar=1.0, in1=d2,
                op0=ALU.mult, op1=ALU.mult, accum_out=sspack[:, 1:2],
            )
            csreg = fin_p.tile([P, 2], fp32, tag="csreg")
            nc.scalar.activation(out=csreg, in_=sspack, func=ACTF.Sqrt)
            ct_ = fin_p.tile([P, 1], fp32, tag="cs_term")
            nc.vector.reduce_sum(out=ct_, in_=csreg, axis=AX.X)
            return ct_


        cs_term = None

        # interleave W builds with the matmul groups they unlock
        if halves:
            for v in range(NV):
                build_w_half(v, 0)
                for a in range(v + 1):
                    mm_group(2 * v, a)
                for n in range(2 * v):
                    mm_group(n, v)
                build_w_half(v, 1)
                for a in range(v + 1):
                    mm_group(2 * v + 1, a)
        elif psum2:
            for v in range(NV):
                build_w(v)
                if v == NV - 1 and cs_early:
                    cs_term = emit_cs()
                for vv in range(v + 1):
                    for a in range(v + 1):
                        if max(vv, a) == v:
                            mm_group2(vv, a)
        elif builds_first:
            for v in range(NV):
                build_w(v)
            for n in range(2 * NV):
                for a in range(NV):
                    mm_group(n, a)
        else:
            for v in range(NV):
                build_w(v)
                for n in range(2 * v + 2):
                    for a in range(v + 1):
                        if max(n // 2, a) == v:
                            mm_group(n, a)

        if cs_term is None:
            cs_term = emit_cs()

        # ---- final reduction ----
        logpack = fin_p.tile([P, 2 * NV], fp32, tag="logpack")
        if psum2:
            rsums = fin_p.tile([P, NV], fp32, tag="rsums")
            nc.vector.reduce_sum(out=rsums, in_=rp_all, axis=AX.X)
            s4s = fin_p.tile([P, NV], fp32, tag="s4s")
            nc.vector.reduce_sum(
                out=s4s,
                in_=dvals.rearrange("p (a b) -> p a b", a=NV),
                axis=AX.X,
            )
            # self terms dvals[:, a*NV+a]: stride-(NV+1) diagonal view
            dd = bass.AP(
                tensor=dvals.tensor, offset=dvals.offset,
                ap=[dvals.ap[0], [NV + 1, NV]],
            )
            nc.vector.tensor_sub(logpack[:, 0:NV], rsums, dd)  # denom
            nc.vector.tensor_sub(logpack[:, NV : 2 * NV], s4s, dd)  # pos
        else:
            for a in range(NV):
                rs = fin_p.tile([P, 1], fp32, tag=f"rs{a}", name=f"rs{a}")
                nc.vector.reduce_sum(out=rs, in_=rowparts[a], axis=AX.X)
                s4 = fin_p.tile([P, 1], fp32, tag=f"s4{a}", name=f"s4{a}")
                nc.vector.reduce_sum(
                    out=s4, in_=dvals[:, a * NV : (a + 1) * NV], axis=AX.X
                )
                da = dvals[:, a * NV + a : a * NV + a + 1]
                nc.vector.tensor_sub(logpack[:, a : a + 1], rs, da)  # denom
                nc.vector.tensor_sub(logpack[:, NV + a : NV + a + 1], s4, da)  # pos
        logs = fin_p.tile([P, 2 * NV], fp32, tag="logs")
        nc.scalar.activation(out=logs, in_=logpack, func=ACTF.Ln)
        s1 = fin_p.tile([P, 1], fp32, tag="s1")
        nc.vector.reduce_sum(out=s1, in_=logs[:, 0:NV], axis=AX.X)
        s2 = fin_p.tile([P, 1], fp32, tag="s2")
        nc.vector.reduce_sum(out=s2, in_=logs[:, NV : 2 * NV], axis=AX.X)
        contrib = fin_p.tile([P, 1], fp32, tag="contrib")
        nc.vector.tensor_sub(contrib, s1, s2)
        out_sb = fin_p.tile([P, 1], fp32, tag="out_sb")
        nc.vector.scalar_tensor_tensor(
            out=out_sb, in0=cs_term, scalar=0.5, in1=contrib,
            op0=ALU.mult, op1=ALU.add,
        )
        nc.sync.dma_start(out=out_d, in_=out_sb)

    nc.compile()
    return nc


def _get_nc():
    if "nc" not in _compiled:
        import os
        _compiled["nc"] = _build_kernel(
            fp8=EMB_FP8, drow=EMB_FP8, cast_sq=EMB_FP8,
            psum2=True, ps_bufs=3, nrm_bufs=2, cs_eng="gpsimd",
            cs_early=True,
        )
    return _compiled["nc"]


EMB_FP8 = True


def _make_in_maps(english, etok, ktoe, korean, cs_ratios):
    e = np.asarray(english, dtype=np.float32)
    etk = np.asarray(etok, dtype=np.float32)
    kte = np.asarray(ktoe, dtype=np.float32)
    k = np.asarray(korean, dtype=np.float32)
    r = np.asarray(cs_ratios, dtype=np.float32)

    # version order must match the reference stack: [e, k, etk, kte]
    V4f = np.stack([e, k, etk, kte])  # [4, B, D] fp32
    emb_np_dt = ml_dtypes.float8_e4m3 if EMB_FP8 else ml_dtypes.bfloat16
    eye = np.eye(P, dtype=np.float32)

    in_maps = []
    for c in range(NC_CORES):
        rot = np.roll(V4f, -c * CHUNK, axis=1)  # [4, B, D], own chunk first
        embT = np.ascontiguousarray(rot.transpose(0, 2, 1)).reshape(NV * D, B).astype(emb_np_dt)
        csrows = np.ascontiguousarray(rot[:, :P, :]).reshape(NV * P, D).astype(ml_dtypes.bfloat16)
        rr = np.roll(r, -c * CHUNK)[:P].reshape(P, 1).astype(np.float32)
        in_maps.append(
            {"embT": embT, "csrows": csrows, "ratios": rr, "eye": eye}
        )
    return in_maps


def kernel(english, etok, ktoe, korean, cs_ratios):
    from concourse.bass_utils import run_bass_kernel_spmd

    in_maps = _make_in_maps(english, etok, ktoe, korean, cs_ratios)
    nc = _get_nc()
    res = run_bass_kernel_spmd(nc, in_maps, core_ids=list(range(NC_CORES)))
    total = 0.0
    for rmap in res.results:
        total += rmap["out"].astype(np.float64).sum()
    return np.array(total / B, dtype=np.float32)



# revision 4
# speedup vs baseline: 1.6739x; 1.6739x over previous
"""CodeSwitchLoss Trainium2 kernel (8-core data-parallel).

Math (see reference): V = l2norm rows of the stack [e, k, etk, kte] (4096 x 1024),
S = V @ V.T, E = exp(10*S).
Per anchor row r=(a,i):
  rowsum[r]   = sum_c E[r,c]
  d_b[r]      = E[r, col(b,i)]  (same-sample entries, b=0..3)
  pos[r]      = sum_{b != a} d_b[r]
  denom[r]    = rowsum[r] - d_a[r]          (= pos + neg)
  contrastive = log(denom) - log(pos)
plus cs regularization on normalized rows; total = (sum contrastive + 0.5*sum reg)/B.

Sharding: batch samples split 8 ways. Each core gets the full embedding set,
rolled so its own 128 samples come first; it computes the 512 anchor rows
(4 versions x 128 samples) against all 4096 columns. Scalar partials summed on
host. The roll makes all per-core slice offsets compile-time constants, so one
NEFF serves all 8 cores.

The host ships rows already l2-normalized (scaled by 32 so fp8e4m3 keeps its
relative precision) and pre-transposed to the matmul layout, so the device does
no norm computation at all: fp8 DoubleRow matmuls into PSUM, one exp per
4-bank group (scale folds in the 10/32^2 temperature factor), DVE rowsum
reduces + eye-masked diagonal extraction, and a short log/sqrt tail. The
measured diagonal is subtracted from both rowsum and pos, so the fp8 norm
noise on the huge self term cancels exactly.
"""

import numpy as np
import ml_dtypes

B = 1024
D = 1024
P = 128
NV = 4
NC_CORES = 8
CHUNK = B // NC_CORES  # 128 samples per core
KCH = D // P  # 8 k-chunks
NT = 512  # matmul free-dim tile (one PSUM bank)
SCALE = 32.0  # fp8 pre-scale on normalized rows
EXPS = 10.0 / (SCALE * SCALE)  # exp scale: 1/T divided by SCALE^2

_compiled = {}


def _sched():
    """Pair-half processing order matching W-version DMA arrival.

    Returns a list of (a, v, h): anchor version a (stationary, own 128
    samples), moving version v, column half h. (a,v,h) needs W[v] half h
    and W[a] cols 0:128 (inside h0)."""
    s = []
    for v in range(NV):
        for a in range(v + 1):
            s.append((a, v, 0))
        for b in range(v):
            s.append((v, b, 0))
            s.append((v, b, 1))
        for a in range(v + 1):
            s.append((a, v, 1))
    return s


def _build_kernel(warm=24, warm_free=128, groups_of=4, drow=True,
                  delay_slot=4, e2_fp32=False):
    from contextlib import ExitStack

    import concourse.bass as bass
    import concourse.tile as tile
    from concourse import bacc, mybir

    fp32 = mybir.dt.float32
    bf16 = mybir.dt.bfloat16
    fp8 = mybir.dt.float8e4
    AX = mybir.AxisListType
    ALU = mybir.AluOpType
    ACTF = mybir.ActivationFunctionType

    nc = bacc.Bacc(
        "TRN2",
        target_bir_lowering=False,
        debug=False,
        enable_asserts=False,
        num_devices=NC_CORES,
    )
    # pre-transposed normalized*32 fp8 embeddings: embT[v*D + d, s] = W_v[s, d]
    embT = nc.dram_tensor("embT", [NV * D, B], fp8, kind="ExternalInput").ap()
    # natural-layout normalized rows of this core's own chunk (for cs reg)
    csrows = nc.dram_tensor("csrows", [NV * P, D], bf16, kind="ExternalInput").ap()
    ratios = nc.dram_tensor("ratios", [P, 1], fp32, kind="ExternalInput").ap()
    eye_d = nc.dram_tensor("eye", [P, P], bf16, kind="ExternalInput").ap()
    out_d = nc.dram_tensor("out", [P, 1], fp32, kind="ExternalOutput").ap()

    sched = _sched()
    ngroups = len(sched) // groups_of
    # h0 slot index per pair (for diag extraction)
    h0_slot = {}
    for slot, (a, v, h) in enumerate(sched):
        if h == 0:
            h0_slot[(a, v)] = slot

    with tile.TileContext(nc) as tc, ExitStack() as ctx:
        consts = ctx.enter_context(tc.tile_pool(name="consts", bufs=1))
        wpool = ctx.enter_context(tc.tile_pool(name="w", bufs=1))
        psum_p = ctx.enter_context(tc.tile_pool(name="psum", bufs=2, space="PSUM"))
        esb_p = ctx.enter_context(tc.tile_pool(name="esb", bufs=3))
        csx_p = ctx.enter_context(tc.tile_pool(name="csx", bufs=1))
        scr_p = ctx.enter_context(tc.tile_pool(name="scr", bufs=2))
        dscr_p = ctx.enter_context(tc.tile_pool(name="dscr", bufs=3))
        fin_p = ctx.enter_context(tc.tile_pool(name="fin", bufs=1))

        # --- constants / warmup (PE busy from t=0 so the clock is ramped
        # to max by the time the first real matmul's W tile has landed) ---
        ones_sb = consts.tile([P, warm_free], bf16, tag="ones")
        nc.vector.memset(ones_sb, 1.0)
        eye_sb = consts.tile([P, P], bf16, tag="eye")
        nc.gpsimd.dma_start(out=eye_sb, in_=eye_d)
        r_sb = consts.tile([P, 1], fp32, tag="ratios")
        nc.gpsimd.dma_start(out=r_sb, in_=ratios)

        W = [
            wpool.tile([P, KCH, B], fp8, tag=f"w{v}", name=f"w{v}")
            for v in range(NV)
        ]

        # --- input DMAs (SP/HWDGE, in arrival order the schedule expects).
        # W3's stationary slice (cols 0:128) is pulled forward so (3,b,*)
        # pairs unlock as soon as their moving version is resident.
        def w_dma(v, c0, c1):
            nc.sync.dma_start(
                out=W[v][:, :, c0:c1],
                in_=embT[v * D : (v + 1) * D, c0:c1].rearrange(
                    "(mm p) s -> p mm s", p=P
                ),
            )

        w_dma(0, 0, NT)
        w_dma(3, 0, P)
        w_dma(0, NT, B)
        w_dma(1, 0, NT)
        w_dma(1, NT, B)
        w_dma(2, 0, NT)
        w_dma(2, NT, B)
        w_dma(3, P, NT)
        w_dma(3, NT, B)

        # warmup matmuls: no data deps beyond the memset; overwritten later
        warm_ps = psum_p.tile([P, groups_of, NT], fp32, tag="ps", name="ps_w")
        for i in range(warm):
            nc.tensor.matmul(
                warm_ps[:, i % groups_of, 0:warm_free],
                ones_sb, ones_sb, start=True, stop=True,
            )

        # --- main loop: groups of `groups_of` pair-halves ---
        e2_dt = fp32 if e2_fp32 else bf16
        rsall = fin_p.tile([P, NV, len(sched) // NV], fp32, tag="rsall")
        rs_idx = {a: 0 for a in range(NV)}
        dvals = fin_p.tile([P, NV * NV], fp32, tag="dvals")  # [:, a*NV + v]

        for g in range(ngroups):
            chunk = sched[g * groups_of : (g + 1) * groups_of]
            ps = psum_p.tile([P, groups_of, NT], fp32, tag="ps", name="ps")
            for j, (a, v, h) in enumerate(chunk):
                if drow:
                    for m in range(0, KCH, 2):
                        nc.tensor.matmul(
                            ps[:, j, :],
                            W[a][:, m : m + 2, 0:P],
                            W[v][:, m : m + 2, h * NT : (h + 1) * NT],
                            start=(m == 0),
                            stop=(m == KCH - 2),
                            perf_mode=mybir.MatmulPerfMode.DoubleRow,
                        )
                else:
                    for m in range(KCH):
                        nc.tensor.matmul(
                            ps[:, j, :],
                            W[a][:, m, 0:P],
                            W[v][:, m, h * NT : (h + 1) * NT],
                            start=(m == 0),
                            stop=(m == KCH - 1),
                        )
            e2 = esb_p.tile([P, groups_of, NT], e2_dt, tag="e2", name="e2")
            nc.scalar.activation(
                out=e2.rearrange("p a b -> p (a b)"),
                in_=ps.rearrange("p a b -> p (a b)"),
                func=ACTF.Exp, scale=EXPS,
            )
            for j, (a, v, h) in enumerate(chunk):
                nc.vector.reduce_sum(
                    out=rsall[:, a, rs_idx[a] : rs_idx[a] + 1],
                    in_=e2[:, j, :], axis=AX.X,
                )
                rs_idx[a] += 1
                if h == 0:
                    dscr = dscr_p.tile([P, P], e2_dt, tag="dscr", name="dscr")
                    nc.vector.scalar_tensor_tensor(
                        out=dscr, in0=e2[:, j, 0:P], scalar=1.0, in1=eye_sb,
                        op0=ALU.mult, op1=ALU.mult,
                        accum_out=dvals[:, a * NV + v : a * NV + v + 1],
                    )
            if g == (delay_slot if delay_slot is not None else -1):
                # csrows DMA deliberately issued mid-kernel from the idle Pool
                # engine so its transfer doesn't steal DMA bandwidth from the
                # W prologue. The copy pins the issue behind group-g's exp.
                dly = fin_p.tile([P, 1], fp32, tag="dly")
                nc.gpsimd.tensor_copy(dly, e2[:, 0, 0:1])
                csx = csx_p.tile([P, NV, D], bf16, tag="csx")
                nc.gpsimd.dma_start(
                    out=csx, in_=csrows.rearrange("(v p) d -> p v d", p=P)
                )

        if delay_slot is None:
            csx = csx_p.tile([P, NV, D], bf16, tag="csx")
            nc.gpsimd.dma_start(
                out=csx, in_=csrows.rearrange("(v p) d -> p v d", p=P)
            )

        # ---- cs regularization on own chunk (rows pre-normalized) ----
        e0, k0, etk0, kte0 = (csx[:, vv, :] for vv in range(NV))
        sspack = fin_p.tile([P, 2], fp32, tag="sspack")
        t1 = scr_p.tile([P, D], bf16, tag="cs_t")
        nc.vector.tensor_sub(t1, e0, k0)
        u = scr_p.tile([P, D], bf16, tag="cs_u")
        nc.vector.tensor_scalar_mul(u, t1, r_sb)
        d1 = scr_p.tile([P, D], bf16, tag="cs_t")
        nc.vector.tensor_sub(d1, etk0, k0)
        d1m = scr_p.tile([P, D], bf16, tag="cs_d")
        nc.vector.tensor_sub(d1m, d1, u)
        dsq1 = scr_p.tile([P, D], bf16, tag="cs_q")
        nc.vector.scalar_tensor_tensor(
            out=dsq1, in0=d1m, scalar=1.0, in1=d1m,
            op0=ALU.mult, op1=ALU.mult, accum_out=sspack[:, 0:1],
        )
        d2 = scr_p.tile([P, D], bf16, tag="cs_t")
        nc.vector.tensor_sub(d2, kte0, e0)
        d2m = scr_p.tile([P, D], bf16, tag="cs_d")
        nc.vector.tensor_add(d2m, d2, u)
        dsq2 = scr_p.tile([P, D], bf16, tag="cs_q")
        nc.vector.scalar_tensor_tensor(
            out=dsq2, in0=d2m, scalar=1.0, in1=d2m,
            op0=ALU.mult, op1=ALU.mult, accum_out=sspack[:, 1:2],
        )
        # sqrt via exp(0.5*ln(x)): stays inside the ln+exp activation table
        lns = fin_p.tile([P, 2], fp32, tag="lns")
        nc.scalar.activation(out=lns, in_=sspack, func=ACTF.Ln)
        csreg = fin_p.tile([P, 2], fp32, tag="csreg")
        nc.scalar.activation(out=csreg, in_=lns, func=ACTF.Exp, scale=0.5)
        cs_term = fin_p.tile([P, 1], fp32, tag="cs_term")
        nc.vector.reduce_sum(out=cs_term, in_=csreg, axis=AX.X)

        # ---- final reduction ----
        rsums = fin_p.tile([P, NV], fp32, tag="rsums")
        nc.vector.reduce_sum(out=rsums, in_=rsall, axis=AX.X)
        s4s = fin_p.tile([P, NV], fp32, tag="s4s")
        nc.vector.reduce_sum(
            out=s4s, in_=dvals.rearrange("p (a b) -> p a b", a=NV), axis=AX.X
        )
        # self terms dvals[:, a*NV+a]: stride-(NV+1) diagonal view
        dd = bass.AP(
            tensor=dvals.tensor, offset=dvals.offset,
            ap=[dvals.ap[0], [NV + 1, NV]],
        )
        logpack = fin_p.tile([P, 2 * NV], fp32, tag="logpack")
        nc.vector.tensor_sub(logpack[:, 0:NV], rsums, dd)  # denom
        nc.vector.tensor_sub(logpack[:, NV : 2 * NV], s4s, dd)  # pos
        logs = fin_p.tile([P, 2 * NV], fp32, tag="logs")
        nc.scalar.activation(out=logs, in_=logpack, func=ACTF.Ln)
        s1 = fin_p.tile([P, 1], fp32, tag="s1")
        nc.vector.reduce_sum(out=s1, in_=logs[:, 0:NV], axis=AX.X)
        s2 = fin_p.tile([P, 1], fp32, tag="s2")
        nc.vector.reduce_sum(out=s2, in_=logs[:, NV : 2 * NV], axis=AX.X)
        contrib = fin_p.tile([P, 1], fp32, tag="contrib")
        nc.vector.tensor_sub(contrib, s1, s2)
        out_sb = fin_p.tile([P, 1], fp32, tag="out_sb")
        nc.vector.scalar_tensor_tensor(
            out=out_sb, in0=cs_term, scalar=0.5, in1=contrib,
            op0=ALU.mult, op1=ALU.add,
        )
        nc.sync.dma_start(out=out_d, in_=out_sb)

    nc.compile()
    return nc


def _get_nc():
    if "nc" not in _compiled:
        _compiled["nc"] = _build_kernel()
    return _compiled["nc"]


def _make_in_maps(english, etok, ktoe, korean, cs_ratios):
    e = np.asarray(english, dtype=np.float32)
    etk = np.asarray(etok, dtype=np.float32)
    kte = np.asarray(ktoe, dtype=np.float32)
    k = np.asarray(korean, dtype=np.float32)
    r = np.asarray(cs_ratios, dtype=np.float32)

    # version order must match the reference stack: [e, k, etk, kte]
    V4f = np.stack([e, k, etk, kte])  # [4, B, D] fp32
    V4n = V4f / np.linalg.norm(V4f, axis=2, keepdims=True)
    V4s = (V4n * SCALE).astype(ml_dtypes.float8_e4m3)
    eye = np.eye(P, dtype=ml_dtypes.bfloat16)

    in_maps = []
    for c in range(NC_CORES):
        rot = np.roll(V4s, -c * CHUNK, axis=1)  # [4, B, D], own chunk first
        embT = np.ascontiguousarray(rot.transpose(0, 2, 1)).reshape(NV * D, B)
        rot_n = np.roll(V4n, -c * CHUNK, axis=1)
        csrows = np.ascontiguousarray(rot_n[:, :P, :]).reshape(NV * P, D).astype(
            ml_dtypes.bfloat16
        )
        rr = np.roll(r, -c * CHUNK)[:P].reshape(P, 1).astype(np.float32)
        in_maps.append(
            {"embT": embT, "csrows": csrows, "ratios": rr, "eye": eye}
        )
    return in_maps


def kernel(english, etok, ktoe, korean, cs_ratios):
    from concourse.bass_utils import run_bass_kernel_spmd

    in_maps = _make_in_maps(english, etok, ktoe, korean, cs_ratios)
    nc = _get_nc()
    res = run_bass_kernel_spmd(nc, in_maps, core_ids=list(range(NC_CORES)))
    total = 0.0
    for rmap in res.results:
        total += rmap["out"].astype(np.float64).sum()
    return np.array(total / B, dtype=np.float32)


# revision 9
# speedup vs baseline: 1.7252x; 1.0307x over previous
"""CodeSwitchLoss Trainium2 kernel (8-core data-parallel).

Math (see reference): V = l2norm rows of the stack [e, k, etk, kte] (4096 x 1024),
S = V @ V.T, E = exp(10*S).
Per anchor row r=(a,i):
  rowsum[r]   = sum_c E[r,c]
  d_b[r]      = E[r, col(b,i)]  (same-sample entries, b=0..3)
  pos[r]      = sum_{b != a} d_b[r]
  denom[r]    = rowsum[r] - d_a[r]          (= pos + neg)
  contrastive = log(denom) - log(pos)
plus cs regularization on normalized rows; total = (sum contrastive + 0.5*sum reg)/B.

Sharding: batch samples split 8 ways. Each core gets the full embedding set,
rolled so its own 128 samples come first; it computes the 512 anchor rows
(4 versions x 128 samples) against all 4096 columns. Scalar partials summed on
host. The roll makes all per-core slice offsets compile-time constants, so one
NEFF serves all 8 cores.

The host ships rows already l2-normalized (scaled by 32 so fp8e4m3 keeps its
relative precision) and pre-transposed to the matmul layout, so the device does
no norm computation at all: fp8 DoubleRow matmuls into PSUM, one exp per
4-bank group (scale folds in the 10/32^2 temperature factor), DVE rowsum
reduces + eye-masked diagonal extraction, and a short log/sqrt tail. The
measured diagonal is subtracted from both rowsum and pos, so the fp8 norm
noise on the huge self term cancels exactly.
"""

import numpy as np
import ml_dtypes

B = 1024
D = 1024
P = 128
NV = 4
NC_CORES = 8
CHUNK = B // NC_CORES  # 128 samples per core
KCH = D // P  # 8 k-chunks
NT = 512  # matmul free-dim tile (one PSUM bank)
SCALE = 32.0  # fp8 pre-scale on normalized rows
EXPS = 10.0 / (SCALE * SCALE)  # exp scale: 1/T divided by SCALE^2

_compiled = {}


def _sched():
    """Pair-half processing order matching W-version DMA arrival.

    Returns a list of (a, v, h): anchor version a (stationary, own 128
    samples), moving version v, column half h. (a,v,h) needs W[v] half h
    and W[a] cols 0:128 (inside h0)."""
    s = []
    for v in range(NV):
        for a in range(v + 1):
            s.append((a, v, 0))
        for b in range(v):
            s.append((v, b, 0))
            s.append((v, b, 1))
        for a in range(v + 1):
            s.append((a, v, 1))
    return s


def _build_kernel(warm=24, warm_free=128, groups_of=2, drow=True,
                  delay_slot=8, e2_fp32=False, ps_bufs=4, rowsum="stt"):
    from contextlib import ExitStack

    import concourse.bass as bass
    import concourse.tile as tile
    from concourse import bacc, mybir

    fp32 = mybir.dt.float32
    bf16 = mybir.dt.bfloat16
    fp8 = mybir.dt.float8e4
    AX = mybir.AxisListType
    ALU = mybir.AluOpType
    ACTF = mybir.ActivationFunctionType

    nc = bacc.Bacc(
        "TRN2",
        target_bir_lowering=False,
        debug=False,
        enable_asserts=False,
        num_devices=NC_CORES,
    )
    # pre-transposed normalized*32 fp8 embeddings: embT[v*D + d, s] = W_v[s, d]
    embT = nc.dram_tensor("embT", [NV * D, B], fp8, kind="ExternalInput").ap()
    # natural-layout normalized rows of this core's own chunk (for cs reg)
    csrows = nc.dram_tensor("csrows", [NV * P, D], bf16, kind="ExternalInput").ap()
    ratios = nc.dram_tensor("ratios", [P, 1], fp32, kind="ExternalInput").ap()
    eye_d = nc.dram_tensor("eye", [P, P], bf16, kind="ExternalInput").ap()
    out_d = nc.dram_tensor("out", [P, 1], fp32, kind="ExternalOutput").ap()

    sched = _sched()
    ngroups = len(sched) // groups_of
    # h0 slot index per pair (for diag extraction)
    h0_slot = {}
    for slot, (a, v, h) in enumerate(sched):
        if h == 0:
            h0_slot[(a, v)] = slot

    with tile.TileContext(nc) as tc, ExitStack() as ctx:
        consts = ctx.enter_context(tc.tile_pool(name="consts", bufs=1))
        wpool = ctx.enter_context(tc.tile_pool(name="w", bufs=1))
        psum_p = ctx.enter_context(
            tc.tile_pool(name="psum", bufs=ps_bufs, space="PSUM")
        )
        esb_p = ctx.enter_context(tc.tile_pool(name="esb", bufs=3))
        csx_p = ctx.enter_context(tc.tile_pool(name="csx", bufs=1))
        scr_p = ctx.enter_context(tc.tile_pool(name="scr", bufs=2))
        dscr_p = ctx.enter_context(tc.tile_pool(name="dscr", bufs=3))
        rsg_p = ctx.enter_context(tc.tile_pool(name="rsg", bufs=2))
        fin_p = ctx.enter_context(tc.tile_pool(name="fin", bufs=1))

        # Pre-load the activation table set holding BOTH Exp and Ln, so the
        # compiler's table-load pass never has to insert a (1.3us) reload
        # between the exp stream and the log/sqrt tail.
        from concourse.hw_specs import get_activation_tables

        tabs = list(get_activation_tables(nc.m.arch).values())
        set_id = next(
            i for i, s in enumerate(tabs)
            if ACTF.Exp in s and ACTF.Ln in s
        )
        nc.scalar.add_instruction(
            mybir.InstLoadActFuncSet(
                name=nc.scalar.bass.get_next_instruction_name(),
                ins=[], outs=[], act_func_set_id=set_id,
            )
        )

        # --- constants / warmup (PE busy from t=0 so the clock is ramped
        # to max by the time the first real matmul's W tile has landed) ---
        ones_sb = consts.tile([P, warm_free], bf16, tag="ones")
        nc.vector.memset(ones_sb, 1.0)
        ones_w = consts.tile([P, groups_of * NT], bf16, tag="ones_w")
        nc.vector.memset(ones_w, 1.0)
        eye_sb = consts.tile([P, P], bf16, tag="eye")
        nc.gpsimd.dma_start(out=eye_sb, in_=eye_d)
        r_sb = consts.tile([P, 1], fp32, tag="ratios")
        nc.gpsimd.dma_start(out=r_sb, in_=ratios)

        W = [
            wpool.tile([P, KCH, B], fp8, tag=f"w{v}", name=f"w{v}")
            for v in range(NV)
        ]

        # --- input DMAs (SP/HWDGE, in arrival order the schedule expects).
        # W3's stationary slice (cols 0:128) is pulled forward so (3,b,*)
        # pairs unlock as soon as their moving version is resident.
        def w_dma(v, c0, c1):
            nc.sync.dma_start(
                out=W[v][:, :, c0:c1],
                in_=embT[v * D : (v + 1) * D, c0:c1].rearrange(
                    "(mm p) s -> p mm s", p=P
                ),
            )

        w_dma(0, 0, NT)
        w_dma(3, 0, P)
        w_dma(0, NT, B)
        w_dma(1, 0, NT)
        w_dma(1, NT, B)
        w_dma(2, 0, NT)
        w_dma(2, NT, B)
        w_dma(3, P, NT)
        w_dma(3, NT, B)

        # warmup matmuls: no data deps beyond the memset; overwritten later
        warm_ps = psum_p.tile([P, groups_of, NT], fp32, tag="ps", name="ps_w")
        for i in range(warm):
            nc.tensor.matmul(
                warm_ps[:, i % groups_of, 0:warm_free],
                ones_sb, ones_sb, start=True, stop=True,
            )

        # --- main loop: groups of `groups_of` pair-halves ---
        e2_dt = fp32 if e2_fp32 else bf16
        rsall = fin_p.tile([P, NV, len(sched) // NV], fp32, tag="rsall")
        nc.vector.memset(rsall, 0.0)  # merged groups leave some slots unused
        rs_idx = {a: 0 for a in range(NV)}
        dvals = fin_p.tile([P, NV * NV], fp32, tag="dvals")  # [:, a*NV + v]

        for g in range(ngroups):
            chunk = sched[g * groups_of : (g + 1) * groups_of]
            ps = psum_p.tile([P, groups_of, NT], fp32, tag="ps", name="ps")
            for j, (a, v, h) in enumerate(chunk):
                if drow:
                    for m in range(0, KCH, 2):
                        nc.tensor.matmul(
                            ps[:, j, :],
                            W[a][:, m : m + 2, 0:P],
                            W[v][:, m : m + 2, h * NT : (h + 1) * NT],
                            start=(m == 0),
                            stop=(m == KCH - 2),
                            perf_mode=mybir.MatmulPerfMode.DoubleRow,
                        )
                else:
                    for m in range(KCH):
                        nc.tensor.matmul(
                            ps[:, j, :],
                            W[a][:, m, 0:P],
                            W[v][:, m, h * NT : (h + 1) * NT],
                            start=(m == 0),
                            stop=(m == KCH - 1),
                        )
            e2 = esb_p.tile([P, groups_of, NT], e2_dt, tag="e2", name="e2")
            nc.scalar.activation(
                out=e2.rearrange("p a b -> p (a b)"),
                in_=ps.rearrange("p a b -> p (a b)"),
                func=ACTF.Exp, scale=EXPS,
            )
            if rowsum == "stt":
                # rowsum partials via STT*ones: runs in the DVE 4x mode
                # (all-bf16 SBUF) with an exact fp32 accumulator, unlike
                # TensorReduce which has no fast mode.
                a0 = chunk[0][0]
                if all(a == a0 for a, _, _ in chunk):
                    rg = rsg_p.tile([P, groups_of * NT], e2_dt, tag="rsg",
                                    name="rsg")
                    nc.vector.scalar_tensor_tensor(
                        out=rg, in0=e2.rearrange("p a b -> p (a b)"),
                        scalar=1.0, in1=ones_w,
                        op0=ALU.mult, op1=ALU.mult,
                        accum_out=rsall[:, a0, rs_idx[a0] : rs_idx[a0] + 1],
                    )
                    rs_idx[a0] += 1
                else:
                    for j, (a, v, h) in enumerate(chunk):
                        rg = rsg_p.tile([P, NT], e2_dt, tag="rsgh", name="rsgh")
                        nc.vector.scalar_tensor_tensor(
                            out=rg, in0=e2[:, j, :], scalar=1.0,
                            in1=ones_w[:, 0:NT],
                            op0=ALU.mult, op1=ALU.mult,
                            accum_out=rsall[:, a, rs_idx[a] : rs_idx[a] + 1],
                        )
                        rs_idx[a] += 1
            else:
                for j, (a, v, h) in enumerate(chunk):
                    nc.vector.reduce_sum(
                        out=rsall[:, a, rs_idx[a] : rs_idx[a] + 1],
                        in_=e2[:, j, :], axis=AX.X,
                    )
                    rs_idx[a] += 1
            for j, (a, v, h) in enumerate(chunk):
                if h == 0:
                    dscr = dscr_p.tile([P, P], e2_dt, tag="dscr", name="dscr")
                    nc.vector.scalar_tensor_tensor(
                        out=dscr, in0=e2[:, j, 0:P], scalar=1.0, in1=eye_sb,
                        op0=ALU.mult, op1=ALU.mult,
                        accum_out=dvals[:, a * NV + v : a * NV + v + 1],
                    )
            if g == (delay_slot if delay_slot is not None else -1):
                # csrows DMA deliberately issued mid-kernel from the idle Pool
                # engine so its transfer doesn't steal DMA bandwidth from the
                # W prologue. The copy pins the issue behind group-g's exp.
                dly = fin_p.tile([P, 1], fp32, tag="dly")
                nc.gpsimd.tensor_copy(dly, e2[:, 0, 0:1])
                csx = csx_p.tile([P, NV, D], bf16, tag="csx")
                nc.gpsimd.dma_start(
                    out=csx, in_=csrows.rearrange("(v p) d -> p v d", p=P)
                )

        if delay_slot is None:
            csx = csx_p.tile([P, NV, D], bf16, tag="csx")
            nc.gpsimd.dma_start(
                out=csx, in_=csrows.rearrange("(v p) d -> p v d", p=P)
            )

        # ---- cs regularization on own chunk (rows pre-normalized) ----
        e0, k0, etk0, kte0 = (csx[:, vv, :] for vv in range(NV))
        sspack = fin_p.tile([P, 2], fp32, tag="sspack")
        t1 = scr_p.tile([P, D], bf16, tag="cs_t")
        nc.vector.tensor_sub(t1, e0, k0)
        u = scr_p.tile([P, D], bf16, tag="cs_u")
        nc.vector.tensor_scalar_mul(u, t1, r_sb)
        d1 = scr_p.tile([P, D], bf16, tag="cs_t")
        nc.vector.tensor_sub(d1, etk0, k0)
        d1m = scr_p.tile([P, D], bf16, tag="cs_d")
        nc.vector.tensor_sub(d1m, d1, u)
        dsq1 = scr_p.tile([P, D], bf16, tag="cs_q")
        nc.vector.scalar_tensor_tensor(
            out=dsq1, in0=d1m, scalar=1.0, in1=d1m,
            op0=ALU.mult, op1=ALU.mult, accum_out=sspack[:, 0:1],
        )
        d2 = scr_p.tile([P, D], bf16, tag="cs_t")
        nc.vector.tensor_sub(d2, kte0, e0)
        d2m = scr_p.tile([P, D], bf16, tag="cs_d")
        nc.vector.tensor_add(d2m, d2, u)
        dsq2 = scr_p.tile([P, D], bf16, tag="cs_q")
        nc.vector.scalar_tensor_tensor(
            out=dsq2, in0=d2m, scalar=1.0, in1=d2m,
            op0=ALU.mult, op1=ALU.mult, accum_out=sspack[:, 1:2],
        )
        # sqrt via exp(0.5*ln(x)): stays inside the ln+exp activation table
        lns = fin_p.tile([P, 2], fp32, tag="lns")
        nc.scalar.activation(out=lns, in_=sspack, func=ACTF.Ln)
        csreg = fin_p.tile([P, 2], fp32, tag="csreg")
        nc.scalar.activation(out=csreg, in_=lns, func=ACTF.Exp, scale=0.5)
        cs_term = fin_p.tile([P, 1], fp32, tag="cs_term")
        nc.vector.reduce_sum(out=cs_term, in_=csreg, axis=AX.X)

        # ---- final reduction ----
        rsums = fin_p.tile([P, NV], fp32, tag="rsums")
        nc.vector.reduce_sum(out=rsums, in_=rsall, axis=AX.X)
        s4s = fin_p.tile([P, NV], fp32, tag="s4s")
        nc.vector.reduce_sum(
            out=s4s, in_=dvals.rearrange("p (a b) -> p a b", a=NV), axis=AX.X
        )
        # self terms dvals[:, a*NV+a]: stride-(NV+1) diagonal view
        dd = bass.AP(
            tensor=dvals.tensor, offset=dvals.offset,
            ap=[dvals.ap[0], [NV + 1, NV]],
        )
        logpack = fin_p.tile([P, 2 * NV], fp32, tag="logpack")
        nc.vector.tensor_sub(logpack[:, 0:NV], rsums, dd)  # denom
        nc.vector.tensor_sub(logpack[:, NV : 2 * NV], s4s, dd)  # pos
        logs = fin_p.tile([P, 2 * NV], fp32, tag="logs")
        nc.scalar.activation(out=logs, in_=logpack, func=ACTF.Ln)
        s1 = fin_p.tile([P, 1], fp32, tag="s1")
        nc.vector.reduce_sum(out=s1, in_=logs[:, 0:NV], axis=AX.X)
        s2 = fin_p.tile([P, 1], fp32, tag="s2")
        nc.vector.reduce_sum(out=s2, in_=logs[:, NV : 2 * NV], axis=AX.X)
        contrib = fin_p.tile([P, 1], fp32, tag="contrib")
        nc.vector.tensor_sub(contrib, s1, s2)
        out_sb = fin_p.tile([P, 1], fp32, tag="out_sb")
        nc.vector.scalar_tensor_tensor(
            out=out_sb, in0=cs_term, scalar=0.5, in1=contrib,
            op0=ALU.mult, op1=ALU.add,
        )
        nc.sync.dma_start(out=out_d, in_=out_sb)

    nc.compile()
    return nc


def _get_nc():
    if "nc" not in _compiled:
        _compiled["nc"] = _build_kernel()
    return _compiled["nc"]


def _make_in_maps(english, etok, ktoe, korean, cs_ratios):
    e = np.asarray(english, dtype=np.float32)
    etk = np.asarray(etok, dtype=np.float32)
    kte = np.asarray(ktoe, dtype=np.float32)
    k = np.asarray(korean, dtype=np.float32)
    r = np.asarray(cs_ratios, dtype=np.float32)

    # version order must match the reference stack: [e, k, etk, kte]
    V4f = np.stack([e, k, etk, kte])  # [4, B, D] fp32
    V4n = V4f / np.linalg.norm(V4f, axis=2, keepdims=True)
    V4s = (V4n * SCALE).astype(ml_dtypes.float8_e4m3)
    eye = np.eye(P, dtype=ml_dtypes.bfloat16)

    in_maps = []
    for c in range(NC_CORES):
        rot = np.roll(V4s, -c * CHUNK, axis=1)  # [4, B, D], own chunk first
        embT = np.ascontiguousarray(rot.transpose(0, 2, 1)).reshape(NV * D, B)
        rot_n = np.roll(V4n, -c * CHUNK, axis=1)
        csrows = np.ascontiguousarray(rot_n[:, :P, :]).reshape(NV * P, D).astype(
            ml_dtypes.bfloat16
        )
        rr = np.roll(r, -c * CHUNK)[:P].reshape(P, 1).astype(np.float32)
        in_maps.append(
            {"embT": embT, "csrows": csrows, "ratios": rr, "eye": eye}
        )
    return in_maps


def kernel(english, etok, ktoe, korean, cs_ratios):
    from concourse.bass_utils import run_bass_kernel_spmd

    in_maps = _make_in_maps(english, etok, ktoe, korean, cs_ratios)
    nc = _get_nc()
    res = run_bass_kernel_spmd(nc, in_maps, core_ids=list(range(NC_CORES)))
    total = 0.0
    for rmap in res.results:
        total += rmap["out"].astype(np.float64).sum()
    return np.array(total / B, dtype=np.float32)
